# revision 1
# baseline (speedup 1.0000x reference)
"""Trainium2 Bass kernel for nn_CRAU (per-channel sparse attention).

Computation (per batch b, channel c):
  qc  = Wq @ src (1x1 conv; bias folded into the S-reduction seed)
  S[c,t] = sum_d unfold(qc)[c,t,d] * feat[c,d] * (1/64)      t in 3x3 window
  A   = softmax_t(S)
  vc  = Wv @ feat + bv (1x1 conv)
  out = fold(A outer vc) * src

Sharding: 8 cores = 4 batches x 2 spatial halves (rows). The q.k reduction
is spatially partial per core; a pairwise AllReduce of S ([128,9] f32 per
channel-half, issued as soon as that half's partials finish so softmax/fold
of one half overlaps the reduction/compute of the other) links the two
halves of each batch.

The fold/unfold (3x3, stride 2, pad 1) is decomposed into 4 output parity
classes, each a small per-channel linear combination of shifted vc planes,
executed with fused DVE scalar_tensor_tensor ops, ScalarE per-partition-
scale multiplies, and GpSimd tensor-tensor multiplies. Convs run on the PE
in fp16 (full rate); the q.k reduction uses the custom-DVE
TENSOR_TENSOR_REDUCE op reading a column-parity-split qc so most of the 9
window offsets stream with unit stride.
"""

import numpy as np

N_CORES = 8
SRC_R, SRC_C = 65, 129          # per-core src slab (padded rows/cols)
FEAT_R, FEAT_C = 33, 66         # per-core feat slab (padded, even width)
SRCN = SRC_R * SRC_C            # 8385
FEATN = FEAT_R * FEAT_C         # 2178
QE_C, QO_C = 66, 64             # qc even-col / odd-col tile widths
OUTN = 64 * 128                 # per-core output elements per channel
SCALE = 1.0 / 64.0
QROWS = 13                      # q-conv rows per PSUM chunk (13*129=1677)

_prog_cache = {}
TRACE = False
TRACE_KW = {}
LAST_RESULT = [None]
STAGE = [99]


def _build(add_bv: bool, stage: int = 99):
    import concourse.mybir as mybir
    import concourse.tile as tile
    from concourse import bacc
    from concourse.dve_ops import TENSOR_TENSOR_REDUCE

    f32 = mybir.dt.float32
    f16 = mybir.dt.float16
    ADD = mybir.AluOpType.add
    MULT = mybir.AluOpType.mult
    MAX = mybir.AluOpType.max
    AX = mybir.AxisListType.X
    Exp = mybir.ActivationFunctionType.Exp

    nc = bacc.Bacc("TRN2", target_bir_lowering=False, debug=False,
                   num_devices=N_CORES)

    src_d = nc.dram_tensor("src", [256, SRCN], f32, kind="ExternalInput").ap()
    feat_d = nc.dram_tensor("feat", [256, FEATN], f32, kind="ExternalInput").ap()
    wpack_d = nc.dram_tensor("wpack", [256, 512], f32, kind="ExternalInput").ap()
    sinit_d = nc.dram_tensor("s_init", [256, 9], f32, kind="ExternalInput").ap()
    bv_d = nc.dram_tensor("bv", [256, 1], f32, kind="ExternalInput").ap()
    out_d = nc.dram_tensor("out", [256, OUTN], f32, kind="ExternalOutput").ap()

    with tile.TileContext(nc) as tc:
        with (
            tc.tile_pool(name="srcp", bufs=2) as srcp,
            tc.tile_pool(name="featp", bufs=2) as featp,
            tc.tile_pool(name="vcp", bufs=2) as vcp,
            tc.tile_pool(name="qcp", bufs=1) as qcp,
            tc.tile_pool(name="constp", bufs=2) as constp,
            tc.tile_pool(name="smp", bufs=1) as smp,
            tc.tile_pool(name="tup", bufs=4) as tup,
            tc.tile_pool(name="outp", bufs=2) as outp,
            tc.tile_pool(name="ps", bufs=2, space="PSUM") as ps,
            tc.tile_pool(name="dramp", bufs=2, space="DRAM") as dramp,
        ):
            # ---- loads (chunked so compute starts early) ----
            src_t = []
            feat_t = []
            w_t = []
            for h in range(2):
                wt = constp.tile([128, 512], f16, tag="w")
                nc.gpsimd.dma_start(wt[:], wpack_d[128 * h:128 * h + 128, :])
                w_t.append(wt)
            for h in range(2):
                st = srcp.tile([128, SRCN], f16, tag="src")
                for c0 in range(0, SRCN, 2145):
                    csz = min(2145, SRCN - c0)
                    nc.gpsimd.dma_start(
                        st[:, c0:c0 + csz],
                        src_d[128 * h:128 * h + 128, c0:c0 + csz])
                src_t.append(st)
                ft = featp.tile([128, FEATN], f16, tag="feat")
                for c0 in range(0, FEATN, 1089):
                    nc.gpsimd.dma_start(
                        ft[:, c0:c0 + 1089],
                        feat_d[128 * h:128 * h + 128, c0:c0 + 1089])
                feat_t.append(ft)

            # smalls layout (cols):
            # [0:9] S(h0) [9:18] S(h1) [18:27] Ssum(h0) [27:36] Ssum(h1)
            # [36:45] A(h0) [45:54] A(h1) [54:63] E scratch
            # [63:64] m [64:65] nm [65:66] sum [66:67] r
            # [68:77] sinit(h0) [77:86] sinit(h1)  [86:88] bv(h0,h1)
            sm = smp.tile([128, 96], f32, tag="smalls")
            for h in range(2):
                nc.sync.dma_start(sm[:, 68 + 9 * h:77 + 9 * h],
                                  sinit_d[128 * h:128 * h + 128, :])
                if add_bv:
                    nc.sync.dma_start(sm[:, 86 + h:87 + h],
                                      bv_d[128 * h:128 * h + 128, :])

            # ---- v-conv (frees PSUM early; vc needed only for fold) ----
            vc_t = []
            for h in range(2 if stage >= 1 else 0):
                vt = vcp.tile([128, FEATN], f16, tag="vc")
                for c0 in range(0, FEATN, 2048):
                    csz = min(2048, FEATN - c0)
                    pt = ps.tile([128, 2048], f32, tag="mm")
                    for s0 in range(0, csz, 512):
                        ssz = min(512, csz - s0)
                        for kt in range(2):
                            nc.tensor.matmul(
                                pt[:, s0:s0 + ssz],
                                lhsT=w_t[kt][:, 256 + 128 * h:256 + 128 * h + 128],
                                rhs=feat_t[kt][:, c0 + s0:c0 + s0 + ssz],
                                start=(kt == 0), stop=(kt == 1))
                    if add_bv:
                        nc.vector.tensor_scalar(
                            out=vt[:, c0:c0 + csz], in0=pt[:, 0:csz],
                            scalar1=sm[:, 86 + h:87 + h], scalar2=None,
                            op0=ADD)
                    else:
                        nc.scalar.copy(vt[:, c0:c0 + csz], pt[:, 0:csz])
                if add_bv:
                    v3 = vt.rearrange("p (r q) -> p r q", q=FEAT_C)
                    nc.gpsimd.memset(v3[:, FEAT_R - 1, :], 0.0)
                    nc.gpsimd.memset(v3[:, :, 64:66], 0.0)
                vc_t.append(vt)

            # ---- q-conv + S partials + per-half collective ----
            S_b = []
            S_r = []
            for h in range(2):
                sbt = dramp.tile([128, 9], f32, tag=f"sb{h}", name=f"sb{h}")
                srt = dramp.tile([128, 9], f32, tag=f"sr{h}", name=f"sr{h}")
                S_b.append(sbt)
                S_r.append(srt)
            for h in range(2 if stage >= 2 else 0):
                # qc column-parity-split tiles:
                #   qe[r, m] = qc[r, 2m]   (m in [0,65), row width QE_C=66)
                #   qo[r, m] = qc[r, 2m+1] (m in [0,64))
                qe = qcp.tile([128, SRC_R * QE_C], f16, tag="qe")
                qo = qcp.tile([128, SRC_R * QO_C], f16, tag="qo")
                qe3 = qe.rearrange("p (r q) -> p r q", q=QE_C)
                qo3 = qo.rearrange("p (r q) -> p r q", q=QO_C)
                # row-aligned PSUM chunks of QROWS src rows each
                for r0 in range(0, SRC_R, QROWS):
                    nrow = min(QROWS, SRC_R - r0)
                    csz = nrow * SRC_C
                    c0 = r0 * SRC_C
                    pt = ps.tile([128, 2048], f32, tag="mm")
                    for s0 in range(0, csz, 512):
                        ssz = min(512, csz - s0)
                        for kt in range(2):
                            nc.tensor.matmul(
                                pt[:, s0:s0 + ssz],
                                lhsT=w_t[kt][:, 128 * h:128 * h + 128],
                                rhs=src_t[kt][:, c0 + s0:c0 + s0 + ssz],
                                start=(kt == 0), stop=(kt == 1))
                    pt3 = pt[:, 0:csz].rearrange("p (r q) -> p r q", q=SRC_C)
                    nc.scalar.copy(qe3[:, r0:r0 + nrow, 0:65],
                                   pt3[:, :, 0:129:2])
                    nc.scalar.copy(qo3[:, r0:r0 + nrow, 0:64],
                                   pt3[:, :, 1:128:2])

                k3 = feat_t[h].rearrange("p (r q) -> p r q", q=FEAT_C)
                scr = tup.tile([128, 2048], f16, tag="tu")
                scr3 = scr.rearrange("p (r q) -> p r q", q=64)
                for i in range(3):
                    for j in range(3):
                        t = 3 * i + j
                        if j == 0:
                            in0 = qe3[:, i:i + 63:2, 0:64]
                        elif j == 2:
                            in0 = qe3[:, i:i + 63:2, 1:65]
                        else:
                            in0 = qo3[:, i:i + 63:2, 0:64]
                        nc.vector._custom_dve(
                            TENSOR_TENSOR_REDUCE,
                            out=scr3[:],
                            in0=in0,
                            in1=k3[:, 0:32, 0:64],
                            s0=sm[:, 68 + 9 * h + t:69 + 9 * h + t],
                            s1=SCALE,
                            accum_out=sm[:, 9 * h + t:9 * h + t + 1])
                nc.sync.dma_start(S_b[h][:], sm[:, 9 * h:9 * h + 9])
                if stage >= 3:
                    nc.gpsimd.collective_compute(
                        "AllReduce", ADD,
                        replica_groups=[[0, 1], [2, 3], [4, 5], [6, 7]],
                        ins=[S_b[h].opt()], outs=[S_r[h].opt()])
                    nc.sync.dma_start(sm[:, 18 + 9 * h:27 + 9 * h], S_r[h][:])

            if stage == 2:
                for h in range(2):
                    nc.sync.dma_start(out_d[128 * h:128 * h + 128, 0:9],
                                      sm[:, 9 * h:9 * h + 9])
            if stage == 3:
                for h in range(2):
                    nc.sync.dma_start(out_d[128 * h:128 * h + 128, 0:9],
                                      sm[:, 18 + 9 * h:27 + 9 * h])

            # ---- softmax + fold + final multiply per half ----
            for h in range(2 if stage >= 4 else 0):
                Ss = sm[:, 18 + 9 * h:27 + 9 * h]
                Av = sm[:, 36 + 9 * h:45 + 9 * h]
                Ev = sm[:, 54:63]
                nc.vector.tensor_reduce(sm[:, 63:64], Ss, axis=AX, op=MAX)
                nc.scalar.mul(sm[:, 64:65], sm[:, 63:64], -1.0)
                nc.scalar.activation(Ev, Ss, Exp, bias=sm[:, 64:65], scale=1.0)
                nc.vector.tensor_reduce(sm[:, 65:66], Ev, axis=AX, op=ADD)
                nc.vector.reciprocal(sm[:, 66:67], sm[:, 65:66])
                nc.vector.tensor_scalar(out=Av, in0=Ev,
                                        scalar1=sm[:, 66:67], scalar2=None,
                                        op0=MULT)

                def a(t):
                    return Av[:, t:t + 1]

                if stage < 5:
                    nc.sync.dma_start(out_d[128 * h:128 * h + 128, 16:25],
                                      sm[:, 36 + 9 * h:45 + 9 * h])
                    continue

                vc3 = vc_t[h].rearrange("p (r q) -> p r q", q=FEAT_C)
                src3 = src_t[h].rearrange("p (r q) -> p r q", q=SRC_C)
                # whole-half views: out rows x in [0,64), v rows m in [0,33)
                v00 = vc3[:, 0:32, 0:64]
                v01 = vc3[:, 0:32, 1:65]
                v10 = vc3[:, 1:33, 0:64]
                v11 = vc3[:, 1:33, 1:65]
                s11 = src3[:, 1:64:2, 1:128:2]
                s12 = src3[:, 1:64:2, 2:129:2]
                s21 = src3[:, 2:65:2, 1:128:2]
                s22 = src3[:, 2:65:2, 2:129:2]

                O = outp.tile([128, OUTN], f32, tag="O")
                O3 = O.rearrange("p (x y) -> p x y", y=128)
                Oee = O3[:, 0:63:2, 0:127:2]
                Oeo = O3[:, 0:63:2, 1:128:2]
                Ooe = O3[:, 1:64:2, 0:127:2]
                Ooo = O3[:, 1:64:2, 1:128:2]

                def v2(tl):
                    return tl.rearrange("p (r q) -> p r q", q=64)

                # ee: (v00 * A4) * src
                nc.vector.scalar_tensor_tensor(
                    out=Oee, in0=v00, scalar=a(4), in1=s11,
                    op0=MULT, op1=MULT)
                # eo: (A3*v01 + A5*v00) * src
                T1 = tup.tile([128, 2048], f16, tag="tu")
                nc.scalar.mul(v2(T1), v00, a(5))
                U1 = tup.tile([128, 2048], f16, tag="tu")
                nc.vector.scalar_tensor_tensor(
                    out=v2(U1), in0=v01, scalar=a(3), in1=v2(T1),
                    op0=MULT, op1=ADD)
                nc.gpsimd.tensor_tensor(out=Oeo, in0=v2(U1), in1=s12, op=MULT)
                # oe: (A1*v10 + A7*v00) * src
                T2 = tup.tile([128, 2048], f16, tag="tu")
                nc.scalar.mul(v2(T2), v10, a(1))
                U2 = tup.tile([128, 2048], f16, tag="tu")
                nc.vector.scalar_tensor_tensor(
                    out=v2(U2), in0=v00, scalar=a(7), in1=v2(T2),
                    op0=MULT, op1=ADD)
                nc.gpsimd.tensor_tensor(out=Ooe, in0=v2(U2), in1=s21, op=MULT)
                # oo: (A0*v11 + A2*v10 + A6*v01 + A8*v00) * src
                T3 = tup.tile([128, 2048], f16, tag="tu")
                nc.scalar.mul(v2(T3), v11, a(0))
                T4 = tup.tile([128, 2048], f16, tag="tu")
                nc.scalar.mul(v2(T4), v01, a(6))
                U3 = tup.tile([128, 2048], f16, tag="tu")
                nc.vector.scalar_tensor_tensor(
                    out=v2(U3), in0=v10, scalar=a(2), in1=v2(T3),
                    op0=MULT, op1=ADD)
                U4 = tup.tile([128, 2048], f16, tag="tu")
                nc.vector.scalar_tensor_tensor(
                    out=v2(U4), in0=v00, scalar=a(8), in1=v2(T4),
                    op0=MULT, op1=ADD)
                U5 = tup.tile([128, 2048], f16, tag="tu")
                nc.vector.tensor_tensor(out=v2(U5), in0=v2(U3), in1=v2(U4),
                                        op=ADD)
                nc.gpsimd.tensor_tensor(out=Ooo, in0=v2(U5), in1=s22, op=MULT)

                nc.sync.dma_start(out_d[128 * h:128 * h + 128, :], O[:])

    nc.compile()
    return nc


def _get_program(add_bv: bool, stage: int = 99):
    key = (add_bv, stage)
    if key not in _prog_cache:
        _prog_cache[key] = _build(add_bv, stage)
    return _prog_cache[key]


def kernel(feat, src, Wq, bq, Wv, bv):
    from concourse.bass_utils import run_bass_kernel_spmd

    feat = np.ascontiguousarray(np.asarray(feat, dtype=np.float32))
    src = np.ascontiguousarray(np.asarray(src, dtype=np.float32))
    Wq = np.asarray(Wq, dtype=np.float32)
    bq = np.asarray(bq, dtype=np.float32)
    Wv = np.asarray(Wv, dtype=np.float32)
    bv = np.asarray(bv, dtype=np.float32)
    B, C, H, W = src.shape

    src_pad = np.pad(src, ((0, 0), (0, 0), (1, 1), (1, 1)))
    feat_pad = np.pad(feat, ((0, 0), (0, 0), (0, 1), (0, 2)))
    wpack = np.ascontiguousarray(
        np.concatenate([Wq.T, Wv.T], axis=1).astype(np.float32))

    add_bv = bool(np.any(bv))
    nc = _get_program(add_bv, STAGE[0])

    # bq correction seeds for the q.k reduction: S += bq * sum(valid k) * scale
    sinits = {}
    if np.any(bq):
        for b in range(B):
            for s in range(2):
                k = feat[b, :, 32 * s:32 * s + 32, :]
                corr = np.zeros((C, 9), np.float32)
                for i in range(3):
                    for j in range(3):
                        valid = np.ones((32, 64), bool)
                        if i == 0 and s == 0:
                            valid[0, :] = False
                        if j == 0:
                            valid[:, 0] = False
                        corr[:, 3 * i + j] = bq * (k * valid).sum((1, 2)) * SCALE
                sinits[(b, s)] = corr
    zero_sinit = np.zeros((C, 9), np.float32)

    in_maps = []
    for core in range(N_CORES):
        b, s = core // 2, core % 2
        src_slab = np.ascontiguousarray(
            src_pad[b, :, 64 * s:64 * s + SRC_R, :SRC_C].reshape(C, SRCN))
        feat_slab = np.ascontiguousarray(
            feat_pad[b, :, 32 * s:32 * s + FEAT_R, :FEAT_C].reshape(C, FEATN))
        in_maps.append({
            "src": src_slab,
            "feat": feat_slab,
            "wpack": wpack,
            "s_init": sinits.get((b, s), zero_sinit),
            "bv": bv.reshape(C, 1),
        })

    res = run_bass_kernel_spmd(nc, in_maps, list(range(N_CORES)),
                               trace=TRACE, **TRACE_KW)
    LAST_RESULT[0] = res

    out = np.empty((B, C, H, W), np.float32)
    for core in range(N_CORES):
        b, s = core // 2, core % 2
        out[b, :, 64 * s:64 * s + 64, :] = \
            res.results[core]["out"].reshape(C, 64, 128)
    return out



# revision 15
# speedup vs baseline: 1.3083x; 1.3083x over previous
"""Trainium2 Bass kernel for nn_CRAU (per-channel sparse attention).

Computation (per batch b, channel c):
  qc  = Wq @ src (1x1 conv; bq folded into the S-reduction seed)
  S[c,t] = sum_d unfold(qc)[c,t,d] * feat[c,d] * (1/64)      t in 3x3 window
  E   = exp(S)  (no max-subtract; S ~ N(0,1)),  r = 1/sum_t E
  vc  = Wv @ feat + bv (1x1 conv)
  out = fold(E outer vc) * r * src

Sharding: 8 cores = 4 batches x 2 spatial halves (rows); pairwise AllReduce
of S ([128,9] f32 per channel-half) links the two halves of each batch.

Layouts: the host pre-converts everything to f16 and splits src into 4
row/col parity planes, so the stride-2 unfold/fold taps become contiguous
plane windows. q-conv runs per plane (conv1x1 is pointwise), the q.k
reduction uses the custom-DVE TENSOR_TENSOR_REDUCE per tap, and the fold
(a per-channel linear combination of shifted vc planes weighted by E_t)
runs on the TensorEngine as diag(E_t) matmuls accumulated in PSUM. The
final (O*r)*src multiply + f16 output conversion runs on GpSimd reading
PSUM directly; output is written as 4 parity planes the host re-interleaves.
"""

import numpy as np

N_CORES = 8
# per-core src slab (padded): 65 rows x 129 cols, split into parity planes
# P_ab[r, c] = slab[2r+a, 2c+b]
PLANES = [(1, 1), (1, 0), (0, 1), (0, 0)]  # load/compute order: oo, oe, eo, ee
PSHAPE = {(0, 0): (33, 65), (0, 1): (33, 64), (1, 0): (32, 65), (1, 1): (32, 64)}
FEAT_R, FEAT_C = 33, 66         # per-core feat slab (padded, even width)
FEATN = FEAT_R * FEAT_C         # 2178
OUTN = 4 * 2048                 # per-core output: 4 parity planes per channel
SCALE = 1.0 / 64.0

# tap t = 3*i + j reads plane (i%2, j%2) at row off i//2, col off j//2.
# class -> (taps, vc views); fold class (a,b) covers out rows 2y+a cols 2x+b
# vc view key: (dr, dc) -> vc[dr:dr+32, dc:dc+64]
CLS_TAPS = {
    (0, 0): [(4, (0, 0))],
    (0, 1): [(3, (0, 1)), (5, (0, 0))],
    (1, 0): [(1, (1, 0)), (7, (0, 0))],
    (1, 1): [(0, (1, 1)), (2, (1, 0)), (6, (0, 1)), (8, (0, 0))],
}
# fold-src view per class: class (a,b) multiplies src plane (1-a, 1-b)
# at row off a, col off b
TAP_PLANE = {t: ((t // 3) % 2, (t % 3) % 2) for t in range(9)}
TAP_OFF = {t: ((t // 3) // 2, (t % 3) // 2) for t in range(9)}

_prog_cache = {}
TRACE = False
TRACE_KW = {}
LAST_RESULT = [None]


DEBUG = [False]


def _build(add_bv: bool):
    debug = DEBUG[0]
    import concourse.mybir as mybir
    import concourse.tile as tile
    from concourse import bacc
    from concourse.dve_ops import TENSOR_TENSOR_REDUCE

    f32 = mybir.dt.float32
    f16 = mybir.dt.float16
    ADD = mybir.AluOpType.add
    MULT = mybir.AluOpType.mult
    AX = mybir.AxisListType.X
    Exp = mybir.ActivationFunctionType.Exp

    nc = bacc.Bacc("TRN2", target_bir_lowering=False, debug=False,
                   num_devices=N_CORES)

    psz = {ab: PSHAPE[ab][0] * PSHAPE[ab][1] for ab in PLANES}
    src_d = {ab: nc.dram_tensor(f"src_{ab[0]}{ab[1]}", [256, psz[ab]], f16,
                                kind="ExternalInput").ap() for ab in PLANES}
    feat_d = nc.dram_tensor("feat", [256, FEATN], f16, kind="ExternalInput").ap()
    wpack_d = nc.dram_tensor("wpack", [256, 512], f16, kind="ExternalInput").ap()
    sinit_d = nc.dram_tensor("s_init", [256, 9], f32, kind="ExternalInput").ap()
    bv_d = nc.dram_tensor("bv", [256, 1], f32, kind="ExternalInput").ap()
    dones_d = nc.dram_tensor("dones", [128, 128], f16, kind="ExternalInput").ap()
    out_d = nc.dram_tensor("out", [256, OUTN], f16, kind="ExternalOutput").ap()
    if debug:
        dbgS_d = nc.dram_tensor("dbgS", [256, 32], f32, kind="ExternalOutput").ap()
        dbgQ_d = nc.dram_tensor("dbgQ", [256, 9 * 2048], f16,
                                kind="ExternalOutput").ap()
        dbgV_d = nc.dram_tensor("dbgV", [256, FEATN], f16,
                                kind="ExternalOutput").ap()
        dbgO_d = nc.dram_tensor("dbgO", [256, OUTN], f16,
                                kind="ExternalOutput").ap()
        dbgF_d = nc.dram_tensor("dbgF", [256, FEATN], f16,
                                kind="ExternalOutput").ap()
        dbgW_d = nc.dram_tensor("dbgW", [256, 512], f16,
                                kind="ExternalOutput").ap()

    with tile.TileContext(nc) as tc:
        with (
            tc.tile_pool(name="srcp", bufs=2) as srcp,
            tc.tile_pool(name="featp", bufs=2) as featp,
            tc.tile_pool(name="vcp", bufs=2) as vcp,
            tc.tile_pool(name="qcp", bufs=2) as qcp,
            tc.tile_pool(name="constp", bufs=2) as constp,
            tc.tile_pool(name="smp", bufs=1) as smp,
            tc.tile_pool(name="scrp", bufs=1) as scrp,
            tc.tile_pool(name="dgp", bufs=2) as dgp,
            tc.tile_pool(name="outp", bufs=4) as outp,
            tc.tile_pool(name="ps", bufs=2, space="PSUM") as ps,
            tc.tile_pool(name="dramp", bufs=2, space="DRAM") as dramp,
        ):
            # ---- loads: weights + diag first, then feat, then src planes ----
            w_t = []
            for kt in range(2):
                wt = constp.tile([128, 512], f16, tag="w")
                nc.sync.dma_start(wt[:], wpack_d[128 * kt:128 * kt + 128, :])
                w_t.append(wt)
            dones = constp.tile([128, 128], f16, tag="dones")
            nc.sync.dma_start(dones[:], dones_d[:, :])

            feat_t = []
            for h in range(2):
                ft = featp.tile([128, FEATN], f16, tag="feat")
                nc.sync.dma_start(ft[:], feat_d[128 * h:128 * h + 128, :])
                feat_t.append(ft)

            if debug:
                for h in range(2):
                    nc.sync.dma_start(dbgF_d[128 * h:128 * h + 128, :],
                                      feat_t[h][:])
                for kt in range(2):
                    nc.sync.dma_start(dbgW_d[128 * kt:128 * kt + 128, :],
                                      w_t[kt][:])

            src_t = []          # src_t[h][ab]
            for h in range(2):
                stl = {}
                for ab in PLANES:
                    st = srcp.tile([128, psz[ab]], f16, tag=f"src{ab}")
                    nc.sync.dma_start(st[:], src_d[ab][128 * h:128 * h + 128, :])
                    stl[ab] = st
                src_t.append(stl)

            # smalls layout (cols):
            # [0:9] S partial  [9:18] Ssum  [18:27] E
            # [27:28] sum  [28:29] r  [30:39] sinit  [40:41] bv
            sm_t = []
            for h in range(2):
                sm = smp.tile([128, 48], f32, tag=f"sm{h}")
                nc.sync.dma_start(sm[:, 30:39], sinit_d[128 * h:128 * h + 128, :])
                if add_bv:
                    nc.sync.dma_start(sm[:, 40:41], bv_d[128 * h:128 * h + 128, :])
                sm_t.append(sm)

            # ---- v-conv (PE; feat arrives first) ----
            vc_t = []
            for h in range(2):
                vt = vcp.tile([128, FEATN], f16, tag="vc")
                for c0 in range(0, FEATN, 2048):
                    csz = min(2048, FEATN - c0)
                    pt = ps.tile([128, 2048], f32, tag="mm")
                    for s0 in range(0, csz, 512):
                        ssz = min(512, csz - s0)
                        for kt in range(2):
                            nc.tensor.matmul(
                                pt[:, s0:s0 + ssz],
                                lhsT=w_t[kt][:, 256 + 128 * h:256 + 128 * h + 128],
                                rhs=feat_t[kt][:, c0 + s0:c0 + s0 + ssz],
                                start=(kt == 0), stop=(kt == 1))
                    if add_bv:
                        nc.vector.tensor_scalar(
                            out=vt[:, c0:c0 + csz], in0=pt[:, 0:csz],
                            scalar1=sm_t[h][:, 40:41], scalar2=None, op0=ADD)
                    else:
                        nc.scalar.copy(vt[:, c0:c0 + csz], pt[:, 0:csz])
                if add_bv:
                    v3 = vt.rearrange("p (r q) -> p r q", q=FEAT_C)
                    nc.gpsimd.memset(v3[:, :, 64:66], 0.0)
                vc_t.append(vt)

            # ---- q-conv per parity plane + TTR + per-half collective ----
            S_b, S_r = [], []
            for h in range(2):
                S_b.append(dramp.tile([128, 9], f32, tag=f"sb{h}", name=f"sb{h}"))
                S_r.append(dramp.tile([128, 9], f32, tag=f"sr{h}", name=f"sr{h}"))

            scr = scrp.tile([128, 2048], f16, tag="ttr_scr")
            scr3 = scr.rearrange("p (r q) -> p r q", q=64)
            # Q planes are stored at 65-wide row stride so TTR tap views
            # never collapse to 2D (in0/in1 must both stay 3D).
            q_t = []            # q_t[h][ab]
            for h in range(2):
                qtl = {}
                for ab in PLANES:
                    rows, wid = PSHAPE[ab]
                    qt = qcp.tile([128, rows * 65], f16, tag=f"q{ab}")
                    qt3 = qt.rearrange("p (r q) -> p r q", q=65)
                    for c0 in range(0, psz[ab], 2048):
                        csz = min(2048, psz[ab] - c0)
                        pt = ps.tile([128, 2048], f32, tag="mm")
                        for s0 in range(0, csz, 512):
                            ssz = min(512, csz - s0)
                            for kt in range(2):
                                nc.tensor.matmul(
                                    pt[:, s0:s0 + ssz],
                                    lhsT=w_t[kt][:, 128 * h:128 * h + 128],
                                    rhs=src_t[kt][ab][:, c0 + s0:c0 + s0 + ssz],
                                    start=(kt == 0), stop=(kt == 1))
                        if wid == 65:
                            nc.scalar.copy(qt[:, c0:c0 + csz], pt[:, 0:csz])
                        else:
                            nr = csz // 64
                            r0 = c0 // 64
                            nc.scalar.copy(
                                qt3[:, r0:r0 + nr, 0:64],
                                pt.rearrange("p (r q) -> p r q", q=64)[:, 0:nr, :])
                    qtl[ab] = qt3
                q_t.append(qtl)

                # TTR taps in plane-arrival order: oo, oe, eo, ee
                k3 = feat_t[h].rearrange("p (r q) -> p r q", q=FEAT_C)
                sm = sm_t[h]
                for t in [4, 3, 5, 1, 7, 0, 2, 6, 8]:
                    ab = TAP_PLANE[t]
                    r0, c0 = TAP_OFF[t]
                    q3 = qtl[ab]
                    nc.vector._custom_dve(
                        TENSOR_TENSOR_REDUCE,
                        out=scr3[:],
                        in0=q3[:, r0:r0 + 32, c0:c0 + 64],
                        in1=k3[:, 0:32, 0:64],
                        s0=sm[:, 30 + t:31 + t],
                        s1=SCALE,
                        accum_out=sm[:, t:t + 1])
                if debug:
                    # dump tap inputs actually seen by TTR
                    for t in range(9):
                        ab = TAP_PLANE[t]
                        r0, c0 = TAP_OFF[t]
                        nc.sync.dma_start(
                            dbgQ_d[128 * h:128 * h + 128,
                                   2048 * t:2048 * t + 2048],
                            qtl[ab][:, r0:r0 + 32, c0:c0 + 64])
                    nc.sync.dma_start(dbgS_d[128 * h:128 * h + 128, 0:9],
                                      sm[:, 0:9])
                nc.sync.dma_start(S_b[h][:], sm[:, 0:9])
                nc.gpsimd.collective_compute(
                    "AllReduce", ADD,
                    replica_groups=[[0, 1], [2, 3], [4, 5], [6, 7]],
                    ins=[S_b[h].opt()], outs=[S_r[h].opt()])
                nc.sync.dma_start(sm[:, 9:18], S_r[h][:])

            # ---- E = exp(S), r = 1/sum; fold on PE; (O*r)*src on GpSimd ----
            for h in range(2):
                sm = sm_t[h]
                Ev = sm[:, 18:27]
                nc.scalar.activation(Ev, sm[:, 9:18], Exp)
                nc.vector.tensor_reduce(sm[:, 27:28], Ev, axis=AX, op=ADD)
                nc.vector.reciprocal(sm[:, 28:29], sm[:, 27:28])
                if debug:
                    nc.sync.dma_start(dbgS_d[128 * h:128 * h + 128, 9:18],
                                      sm[:, 9:18])
                    nc.sync.dma_start(dbgS_d[128 * h:128 * h + 128, 18:27],
                                      sm[:, 18:27])
                    nc.sync.dma_start(dbgS_d[128 * h:128 * h + 128, 27:29],
                                      sm[:, 27:29])
                    nc.sync.dma_start(dbgV_d[128 * h:128 * h + 128, :],
                                      vc_t[h][:])

                # diag(E_t) tiles for the PE fold
                dg = dgp.tile([128, 9 * 128], f16, tag="diag")
                for t in range(9):
                    nc.vector.tensor_scalar(
                        out=dg[:, 128 * t:128 * t + 128], in0=dones[:],
                        scalar1=sm[:, 18 + t:19 + t], scalar2=None, op0=MULT)

                vc3 = vc_t[h].rearrange("p (r q) -> p r q", q=FEAT_C)
                for ci, (a, b) in enumerate(PLANES[::-1]):  # ee, eo, oe, oo
                    taps = CLS_TAPS[(a, b)]
                    pt = ps.tile([128, 2048], f32, tag="mm")
                    pt3 = pt.rearrange("p (r q) -> p r q", q=64)
                    for ti, (t, (dr, dc)) in enumerate(taps):
                        for r0 in range(0, 32, 8):
                            nc.tensor.matmul(
                                pt3[:, r0:r0 + 8, :],
                                lhsT=dg[:, 128 * t:128 * t + 128],
                                rhs=vc3[:, dr + r0:dr + r0 + 8, dc:dc + 64],
                                start=(ti == 0), stop=(ti == len(taps) - 1))
                    # evict O*r to SBUF f16 (Act, scale=r), then U*src TT
                    sab = (1 - a, 1 - b)
                    s3 = src_t[h][sab].rearrange("p (r q) -> p r q",
                                                 q=PSHAPE[sab][1])
                    ut = outp.tile([128, 2048], f16, tag="U")
                    nc.scalar.mul(ut[:], pt[:, 0:2048], sm[:, 28:29])
                    if debug:
                        nc.sync.dma_start(
                            dbgO_d[128 * h:128 * h + 128,
                                   2048 * (2 * a + b):2048 * (2 * a + b) + 2048],
                            ut[:])
                    ot = outp.tile([128, 2048], f16, tag="O")
                    eng = nc.vector if ci == 3 else nc.gpsimd
                    eng.tensor_tensor(
                        out=ot.rearrange("p (r q) -> p r q", q=64),
                        in0=ut.rearrange("p (r q) -> p r q", q=64),
                        in1=s3[:, a:a + 32, b:b + 64], op=MULT)
                    cls = 2 * a + b
                    nc.sync.dma_start(
                        out_d[128 * h:128 * h + 128,
                              2048 * cls:2048 * cls + 2048], ot[:])

    nc.compile()
    return nc


def _get_program(add_bv: bool):
    if add_bv not in _prog_cache:
        _prog_cache[add_bv] = _build(add_bv)
    return _prog_cache[add_bv]


def kernel(feat, src, Wq, bq, Wv, bv):
    from concourse.bass_utils import run_bass_kernel_spmd

    feat = np.ascontiguousarray(np.asarray(feat, dtype=np.float32))
    src = np.ascontiguousarray(np.asarray(src, dtype=np.float32))
    Wq = np.asarray(Wq, dtype=np.float32)
    bq = np.asarray(bq, dtype=np.float32)
    Wv = np.asarray(Wv, dtype=np.float32)
    bv = np.asarray(bv, dtype=np.float32)
    B, C, H, W = src.shape

    src_pad = np.pad(src, ((0, 0), (0, 0), (1, 1), (1, 1))).astype(np.float16)
    feat_pad = np.pad(feat, ((0, 0), (0, 0), (0, 1), (0, 2))).astype(np.float16)
    wpack = np.ascontiguousarray(
        np.concatenate([Wq.T, Wv.T], axis=1)).astype(np.float16)
    dones = np.eye(128, dtype=np.float16)

    add_bv = bool(np.any(bv))
    nc = _get_program(add_bv)

    # bq correction seeds: S += bq * sum(valid k) * scale
    sinits = {}
    if np.any(bq):
        for b in range(B):
            for s in range(2):
                k = feat[b, :, 32 * s:32 * s + 32, :]
                corr = np.zeros((C, 9), np.float32)
                for i in range(3):
                    for j in range(3):
                        valid = np.ones((32, 64), bool)
                        if i == 0 and s == 0:
                            valid[0, :] = False
                        if j == 0:
                            valid[:, 0] = False
                        corr[:, 3 * i + j] = bq * (k * valid).sum((1, 2)) * SCALE
                sinits[(b, s)] = corr
    zero_sinit = np.zeros((C, 9), np.float32)

    in_maps = []
    for core in range(N_CORES):
        b, s = core // 2, core % 2
        slab = src_pad[b, :, 64 * s:64 * s + 65, :129]
        im = {
            "feat": np.ascontiguousarray(
                feat_pad[b, :, 32 * s:32 * s + FEAT_R, :FEAT_C]
            ).reshape(C, FEATN),
            "wpack": wpack,
            "s_init": sinits.get((b, s), zero_sinit),
            "bv": bv.reshape(C, 1).astype(np.float32),
            "dones": dones,
        }
        for (a, bb) in PLANES:
            pr, pc = PSHAPE[(a, bb)]
            im[f"src_{a}{bb}"] = np.ascontiguousarray(
                slab[:, a::2, bb::2]).reshape(C, pr * pc)
        in_maps.append(im)

    res = run_bass_kernel_spmd(nc, in_maps, list(range(N_CORES)),
                               trace=TRACE, **TRACE_KW)
    LAST_RESULT[0] = res

    out = np.empty((B, C, H, W), np.float32)
    for core in range(N_CORES):
        b, s = core // 2, core % 2
        planes = res.results[core]["out"].astype(np.float32).reshape(C, 4, 32, 64)
        for cls, (a, bb) in enumerate([(0, 0), (0, 1), (1, 0), (1, 1)]):
            out[b, :, 64 * s + a:64 * s + 64:2, bb::2] = planes[:, cls]
    return out


# revision 17
# speedup vs baseline: 1.4650x; 1.1198x over previous
"""Trainium2 Bass kernel for nn_CRAU (per-channel sparse attention).

Computation (per batch b, channel c):
  qc  = Wq @ src (1x1 conv; bq folded into the S-reduction seed)
  S[c,t] = sum_d unfold(qc)[c,t,d] * feat[c,d] * (1/64)      t in 3x3 window
  E   = exp(S)  (no max-subtract; S ~ N(0,1)),  r = 1/sum_t E
  vc  = Wv @ feat + bv (1x1 conv)
  out = fold(E outer vc) * r * src

Sharding: 8 cores = 4 batches x 2 spatial halves (rows); pairwise AllReduce
of S ([128,9] f32 per channel-half) links the two halves of each batch.

Layouts: the host pre-converts everything to f16 and splits src into 4
row/col parity planes, so the stride-2 unfold/fold taps become contiguous
plane windows. q-conv runs per plane (conv1x1 is pointwise), the q.k
reduction uses the custom-DVE TENSOR_TENSOR_REDUCE per tap, and the fold
(a per-channel linear combination of shifted vc planes weighted by E_t)
runs on the TensorEngine as diag(E_t) matmuls accumulated in PSUM. The
final (O*r)*src multiply + f16 output conversion runs on GpSimd reading
PSUM directly; output is written as 4 parity planes the host re-interleaves.
"""

import numpy as np

N_CORES = 8
# per-core src slab (padded): 65 rows x 129 cols, split into parity planes
# P_ab[r, c] = slab[2r+a, 2c+b]
PLANES = [(1, 1), (1, 0), (0, 1), (0, 0)]  # load/compute order: oo, oe, eo, ee
PSHAPE = {(0, 0): (33, 65), (0, 1): (33, 64), (1, 0): (32, 65), (1, 1): (32, 64)}
FEAT_R, FEAT_C = 33, 66         # per-core feat slab (padded, even width)
FEATN = FEAT_R * FEAT_C         # 2178
OUTN = 4 * 2048                 # per-core output: 4 parity planes per channel
SCALE = 1.0 / 64.0

# tap t = 3*i + j reads plane (i%2, j%2) at row off i//2, col off j//2.
# class -> (taps, vc views); fold class (a,b) covers out rows 2y+a cols 2x+b
# vc view key: (dr, dc) -> vc[dr:dr+32, dc:dc+64]
CLS_TAPS = {
    (0, 0): [(4, (0, 0))],
    (0, 1): [(3, (0, 1)), (5, (0, 0))],
    (1, 0): [(1, (1, 0)), (7, (0, 0))],
    (1, 1): [(0, (1, 1)), (2, (1, 0)), (6, (0, 1)), (8, (0, 0))],
}
# fold-src view per class: class (a,b) multiplies src plane (1-a, 1-b)
# at row off a, col off b
TAP_PLANE = {t: ((t // 3) % 2, (t % 3) % 2) for t in range(9)}
TAP_OFF = {t: ((t // 3) // 2, (t % 3) // 2) for t in range(9)}

_prog_cache = {}
TRACE = False
TRACE_KW = {}
LAST_RESULT = [None]


DEBUG = [False]


def _build(add_bv: bool):
    debug = DEBUG[0]
    import concourse.mybir as mybir
    import concourse.tile as tile
    from concourse import bacc
    from concourse.dve_ops import TENSOR_TENSOR_REDUCE

    f32 = mybir.dt.float32
    f16 = mybir.dt.float16
    ADD = mybir.AluOpType.add
    MULT = mybir.AluOpType.mult
    AX = mybir.AxisListType.X
    Exp = mybir.ActivationFunctionType.Exp

    nc = bacc.Bacc("TRN2", target_bir_lowering=False, debug=False,
                   num_devices=N_CORES)

    psz = {ab: PSHAPE[ab][0] * PSHAPE[ab][1] for ab in PLANES}
    src_d = {ab: nc.dram_tensor(f"src_{ab[0]}{ab[1]}", [256, psz[ab]], f16,
                                kind="ExternalInput").ap() for ab in PLANES}
    feat_d = nc.dram_tensor("feat", [256, FEATN], f16, kind="ExternalInput").ap()
    wpack_d = nc.dram_tensor("wpack", [256, 512], f16, kind="ExternalInput").ap()
    sinit_d = nc.dram_tensor("s_init", [256, 9], f32, kind="ExternalInput").ap()
    bv_d = nc.dram_tensor("bv", [256, 1], f32, kind="ExternalInput").ap()
    dones_d = nc.dram_tensor("dones", [128, 128], f16, kind="ExternalInput").ap()
    out_d = nc.dram_tensor("out", [256, OUTN], f16, kind="ExternalOutput").ap()
    if debug:
        dbgS_d = nc.dram_tensor("dbgS", [256, 32], f32, kind="ExternalOutput").ap()
        dbgQ_d = nc.dram_tensor("dbgQ", [256, 9 * 2048], f16,
                                kind="ExternalOutput").ap()
        dbgV_d = nc.dram_tensor("dbgV", [256, FEATN], f16,
                                kind="ExternalOutput").ap()
        dbgO_d = nc.dram_tensor("dbgO", [256, OUTN], f16,
                                kind="ExternalOutput").ap()
        dbgF_d = nc.dram_tensor("dbgF", [256, FEATN], f16,
                                kind="ExternalOutput").ap()
        dbgW_d = nc.dram_tensor("dbgW", [256, 512], f16,
                                kind="ExternalOutput").ap()

    with tile.TileContext(nc) as tc:
        with (
            tc.tile_pool(name="srcp", bufs=2) as srcp,
            tc.tile_pool(name="featp", bufs=2) as featp,
            tc.tile_pool(name="vcp", bufs=2) as vcp,
            tc.tile_pool(name="qcp", bufs=2) as qcp,
            tc.tile_pool(name="constp", bufs=2) as constp,
            tc.tile_pool(name="smp", bufs=1) as smp,
            tc.tile_pool(name="scrp", bufs=1) as scrp,
            tc.tile_pool(name="dgp", bufs=2) as dgp,
            tc.tile_pool(name="outp", bufs=4) as outp,
            tc.tile_pool(name="ps", bufs=2, space="PSUM") as ps,
            tc.tile_pool(name="dramp", bufs=2, space="DRAM") as dramp,
        ):
            # ---- loads: w/dones/smalls, src_oo, feat, then other planes ----
            w_t = []
            for kt in range(2):
                wt = constp.tile([128, 512], f16, tag="w")
                nc.sync.dma_start(wt[:], wpack_d[128 * kt:128 * kt + 128, :])
                w_t.append(wt)
            dones = constp.tile([128, 128], f16, tag="dones")
            nc.sync.dma_start(dones[:], dones_d[:, :])

            # smalls layout (cols):
            # [0:9] S partial  [9:18] Ssum  [18:27] E
            # [27:28] sum  [28:29] r  [30:39] sinit  [40:41] bv
            sm_t = []
            for h in range(2):
                sm = smp.tile([128, 48], f32, tag=f"sm{h}")
                nc.sync.dma_start(sm[:, 30:39], sinit_d[128 * h:128 * h + 128, :])
                if add_bv:
                    nc.sync.dma_start(sm[:, 40:41], bv_d[128 * h:128 * h + 128, :])
                sm_t.append(sm)

            src_t = [{}, {}]    # src_t[h][ab]
            feat_t = []

            def load_plane(ab):
                for h in range(2):
                    st = srcp.tile([128, psz[ab]], f16, tag=f"src{ab}")
                    nc.sync.dma_start(st[:], src_d[ab][128 * h:128 * h + 128, :])
                    src_t[h][ab] = st

            load_plane((1, 1))
            for h in range(2):
                ft = featp.tile([128, FEATN], f16, tag="feat")
                nc.sync.dma_start(ft[:], feat_d[128 * h:128 * h + 128, :])
                feat_t.append(ft)
            load_plane((1, 0))
            load_plane((0, 1))
            load_plane((0, 0))

            if debug:
                for h in range(2):
                    nc.sync.dma_start(dbgF_d[128 * h:128 * h + 128, :],
                                      feat_t[h][:])
                for kt in range(2):
                    nc.sync.dma_start(dbgW_d[128 * kt:128 * kt + 128, :],
                                      w_t[kt][:])

            # ---- q-conv per parity plane + v-conv + TTR + collectives ----
            S_b, S_r = [], []
            for h in range(2):
                S_b.append(dramp.tile([128, 9], f32, tag=f"sb{h}", name=f"sb{h}"))
                S_r.append(dramp.tile([128, 9], f32, tag=f"sr{h}", name=f"sr{h}"))

            scr = scrp.tile([128, 2048], f16, tag="ttr_scr")
            scr3 = scr.rearrange("p (r q) -> p r q", q=64)
            q_t = []            # q_t[h][ab]
            vc_t = []

            def qconv_plane(h, ab, qtl):
                # Q planes stored at 65-wide row stride so TTR tap views
                # never collapse to 2D (in0/in1 must both stay 3D).
                rows, wid = PSHAPE[ab]
                qt = qcp.tile([128, rows * 65], f16, tag=f"q{ab}")
                qt3 = qt.rearrange("p (r q) -> p r q", q=65)
                for c0 in range(0, psz[ab], 2048):
                    csz = min(2048, psz[ab] - c0)
                    pt = ps.tile([128, 2048], f32, tag="mm")
                    for s0 in range(0, csz, 512):
                        ssz = min(512, csz - s0)
                        for kt in range(2):
                            nc.tensor.matmul(
                                pt[:, s0:s0 + ssz],
                                lhsT=w_t[kt][:, 128 * h:128 * h + 128],
                                rhs=src_t[kt][ab][:, c0 + s0:c0 + s0 + ssz],
                                start=(kt == 0), stop=(kt == 1))
                    if wid == 65:
                        nc.scalar.copy(qt[:, c0:c0 + csz], pt[:, 0:csz])
                    else:
                        nr = csz // 64
                        r0 = c0 // 64
                        nc.scalar.copy(
                            qt3[:, r0:r0 + nr, 0:64],
                            pt.rearrange("p (r q) -> p r q", q=64)[:, 0:nr, :])
                qtl[ab] = qt3

            def vconv(h):
                vt = vcp.tile([128, FEATN], f16, tag="vc")
                for c0 in range(0, FEATN, 2048):
                    csz = min(2048, FEATN - c0)
                    pt = ps.tile([128, 2048], f32, tag="mm")
                    for s0 in range(0, csz, 512):
                        ssz = min(512, csz - s0)
                        for kt in range(2):
                            nc.tensor.matmul(
                                pt[:, s0:s0 + ssz],
                                lhsT=w_t[kt][:, 256 + 128 * h:256 + 128 * h + 128],
                                rhs=feat_t[kt][:, c0 + s0:c0 + s0 + ssz],
                                start=(kt == 0), stop=(kt == 1))
                    if add_bv:
                        nc.vector.tensor_scalar(
                            out=vt[:, c0:c0 + csz], in0=pt[:, 0:csz],
                            scalar1=sm_t[h][:, 40:41], scalar2=None, op0=ADD)
                    else:
                        nc.scalar.copy(vt[:, c0:c0 + csz], pt[:, 0:csz])
                if add_bv:
                    v3 = vt.rearrange("p (r q) -> p r q", q=FEAT_C)
                    nc.gpsimd.memset(v3[:, :, 64:66], 0.0)
                vc_t.append(vt)

            for h in range(2):
                qtl = {}
                qconv_plane(h, (1, 1), qtl)
                qconv_plane(h, (1, 0), qtl)
                qconv_plane(h, (0, 1), qtl)
                vconv(h)
                qconv_plane(h, (0, 0), qtl)
                q_t.append(qtl)

                # TTR taps in plane-arrival order: oo, oe, eo, ee
                k3 = feat_t[h].rearrange("p (r q) -> p r q", q=FEAT_C)
                sm = sm_t[h]
                for t in [4, 3, 5, 1, 7, 0, 2, 6, 8]:
                    ab = TAP_PLANE[t]
                    r0, c0 = TAP_OFF[t]
                    q3 = qtl[ab]
                    nc.vector._custom_dve(
                        TENSOR_TENSOR_REDUCE,
                        out=scr3[:],
                        in0=q3[:, r0:r0 + 32, c0:c0 + 64],
                        in1=k3[:, 0:32, 0:64],
                        s0=sm[:, 30 + t:31 + t],
                        s1=SCALE,
                        accum_out=sm[:, t:t + 1])
                if debug:
                    for t in range(9):
                        ab = TAP_PLANE[t]
                        r0, c0 = TAP_OFF[t]
                        nc.sync.dma_start(
                            dbgQ_d[128 * h:128 * h + 128,
                                   2048 * t:2048 * t + 2048],
                            qtl[ab][:, r0:r0 + 32, c0:c0 + 64])
                    nc.sync.dma_start(dbgS_d[128 * h:128 * h + 128, 0:9],
                                      sm[:, 0:9])
                nc.sync.dma_start(S_b[h][:], sm[:, 0:9])
                nc.gpsimd.collective_compute(
                    "AllReduce", ADD,
                    replica_groups=[[0, 1], [2, 3], [4, 5], [6, 7]],
                    ins=[S_b[h].opt()], outs=[S_r[h].opt()])

            # ---- E = exp(S), r = 1/sum; fold on PE; (O*r)*src on V/G ----
            for h in range(2):
                sm = sm_t[h]
                nc.sync.dma_start(sm[:, 9:18], S_r[h][:])
                Ev = sm[:, 18:27]
                nc.scalar.activation(Ev, sm[:, 9:18], Exp)
                nc.vector.tensor_reduce(sm[:, 27:28], Ev, axis=AX, op=ADD)
                nc.vector.reciprocal(sm[:, 28:29], sm[:, 27:28])
                if debug:
                    nc.sync.dma_start(dbgS_d[128 * h:128 * h + 128, 9:18],
                                      sm[:, 9:18])
                    nc.sync.dma_start(dbgS_d[128 * h:128 * h + 128, 18:27],
                                      sm[:, 18:27])
                    nc.sync.dma_start(dbgS_d[128 * h:128 * h + 128, 27:29],
                                      sm[:, 27:29])
                    nc.sync.dma_start(dbgV_d[128 * h:128 * h + 128, :],
                                      vc_t[h][:])

                # diag(E_t) tiles for the PE fold
                dg = dgp.tile([128, 9 * 128], f16, tag="diag")
                for t in range(9):
                    nc.vector.tensor_scalar(
                        out=dg[:, 128 * t:128 * t + 128], in0=dones[:],
                        scalar1=sm[:, 18 + t:19 + t], scalar2=None, op0=MULT)

                vc3 = vc_t[h].rearrange("p (r q) -> p r q", q=FEAT_C)
                for ci, (a, b) in enumerate(PLANES[::-1]):  # ee, eo, oe, oo
                    taps = CLS_TAPS[(a, b)]
                    on_v = ci in (1, 3)  # eo, oo -> Vector; ee, oe -> GpSimd
                    pt = ps.tile([128, 2048], f32, tag="mm")
                    pt3 = pt.rearrange("p (r q) -> p r q", q=64)
                    for ti, (t, (dr, dc)) in enumerate(taps):
                        for r0 in range(0, 32, 8):
                            nc.tensor.matmul(
                                pt3[:, r0:r0 + 8, :],
                                lhsT=dg[:, 128 * t:128 * t + 128],
                                rhs=vc3[:, dr + r0:dr + r0 + 8, dc:dc + 64],
                                start=(ti == 0), stop=(ti == len(taps) - 1))
                    # evict O*r to SBUF f16 (Act, scale=r), then U*src TT.
                    # Vector classes use 65-stride tiles so all TT operands
                    # are matched 3D views (keeps DVE in fast mode).
                    sab = (1 - a, 1 - b)
                    s3 = src_t[h][sab].rearrange("p (r q) -> p r q",
                                                 q=PSHAPE[sab][1])
                    cls = 2 * a + b
                    if on_v:
                        ut = outp.tile([128, 32 * 65], f16, tag="Uv")
                        ut3 = ut.rearrange("p (r q) -> p r q", q=65)
                        nc.scalar.mul(ut3[:, 0:32, 0:64], pt3[:], sm[:, 28:29])
                        ot = outp.tile([128, 32 * 65], f16, tag="Ov")
                        ot3 = ot.rearrange("p (r q) -> p r q", q=65)
                        nc.vector.tensor_tensor(
                            out=ot3[:, 0:32, 0:64], in0=ut3[:, 0:32, 0:64],
                            in1=s3[:, a:a + 32, b:b + 64], op=MULT)
                        nc.sync.dma_start(
                            out_d[128 * h:128 * h + 128,
                                  2048 * cls:2048 * cls + 2048],
                            ot3[:, 0:32, 0:64])
                    else:
                        ut = outp.tile([128, 2048], f16, tag="U")
                        nc.scalar.mul(ut[:], pt[:, 0:2048], sm[:, 28:29])
                        ot = outp.tile([128, 2048], f16, tag="O")
                        nc.gpsimd.tensor_tensor(
                            out=ot.rearrange("p (r q) -> p r q", q=64),
                            in0=ut.rearrange("p (r q) -> p r q", q=64),
                            in1=s3[:, a:a + 32, b:b + 64], op=MULT)
                        nc.sync.dma_start(
                            out_d[128 * h:128 * h + 128,
                                  2048 * cls:2048 * cls + 2048], ot[:])
                    if debug:
                        uview = ut3[:, 0:32, 0:64] if on_v else ut[:]
                        nc.sync.dma_start(
                            dbgO_d[128 * h:128 * h + 128,
                                   2048 * cls:2048 * cls + 2048], uview)

    nc.compile()
    return nc


def _get_program(add_bv: bool):
    if add_bv not in _prog_cache:
        _prog_cache[add_bv] = _build(add_bv)
    return _prog_cache[add_bv]


def kernel(feat, src, Wq, bq, Wv, bv):
    from concourse.bass_utils import run_bass_kernel_spmd

    feat = np.ascontiguousarray(np.asarray(feat, dtype=np.float32))
    src = np.ascontiguousarray(np.asarray(src, dtype=np.float32))
    Wq = np.asarray(Wq, dtype=np.float32)
    bq = np.asarray(bq, dtype=np.float32)
    Wv = np.asarray(Wv, dtype=np.float32)
    bv = np.asarray(bv, dtype=np.float32)
    B, C, H, W = src.shape

    src_pad = np.pad(src, ((0, 0), (0, 0), (1, 1), (1, 1))).astype(np.float16)
    feat_pad = np.pad(feat, ((0, 0), (0, 0), (0, 1), (0, 2))).astype(np.float16)
    wpack = np.ascontiguousarray(
        np.concatenate([Wq.T, Wv.T], axis=1)).astype(np.float16)
    dones = np.eye(128, dtype=np.float16)

    add_bv = bool(np.any(bv))
    nc = _get_program(add_bv)

    # bq correction seeds: S += bq * sum(valid k) * scale
    sinits = {}
    if np.any(bq):
        for b in range(B):
            for s in range(2):
                k = feat[b, :, 32 * s:32 * s + 32, :]
                corr = np.zeros((C, 9), np.float32)
                for i in range(3):
                    for j in range(3):
                        valid = np.ones((32, 64), bool)
                        if i == 0 and s == 0:
                            valid[0, :] = False
                        if j == 0:
                            valid[:, 0] = False
                        corr[:, 3 * i + j] = bq * (k * valid).sum((1, 2)) * SCALE
                sinits[(b, s)] = corr
    zero_sinit = np.zeros((C, 9), np.float32)

    in_maps = []
    for core in range(N_CORES):
        b, s = core // 2, core % 2
        slab = src_pad[b, :, 64 * s:64 * s + 65, :129]
        im = {
            "feat": np.ascontiguousarray(
                feat_pad[b, :, 32 * s:32 * s + FEAT_R, :FEAT_C]
            ).reshape(C, FEATN),
            "wpack": wpack,
            "s_init": sinits.get((b, s), zero_sinit),
            "bv": bv.reshape(C, 1).astype(np.float32),
            "dones": dones,
        }
        for (a, bb) in PLANES:
            pr, pc = PSHAPE[(a, bb)]
            im[f"src_{a}{bb}"] = np.ascontiguousarray(
                slab[:, a::2, bb::2]).reshape(C, pr * pc)
        in_maps.append(im)

    res = run_bass_kernel_spmd(nc, in_maps, list(range(N_CORES)),
                               trace=TRACE, **TRACE_KW)
    LAST_RESULT[0] = res

    out = np.empty((B, C, H, W), np.float32)
    for core in range(N_CORES):
        b, s = core // 2, core % 2
        planes = res.results[core]["out"].astype(np.float32).reshape(C, 4, 32, 64)
        for cls, (a, bb) in enumerate([(0, 0), (0, 1), (1, 0), (1, 1)]):
            out[b, :, 64 * s + a:64 * s + 64:2, bb::2] = planes[:, cls]
    return out


# revision 21
# speedup vs baseline: 1.5125x; 1.0324x over previous
"""Trainium2 Bass kernel for nn_CRAU (per-channel sparse attention).

Computation (per batch b, channel c):
  qc  = Wq @ src (1x1 conv; bq folded into the S-reduction seed)
  S[c,t] = sum_d unfold(qc)[c,t,d] * feat[c,d] * (1/64)      t in 3x3 window
  E   = exp(S)  (no max-subtract; S ~ N(0,1)),  r = 1/sum_t E
  vc  = Wv @ feat + bv (1x1 conv)
  out = fold(E outer vc) * r * src

Sharding: 8 cores = 4 batches x 2 spatial halves (rows); pairwise AllReduce
of S ([128,9] f32 per channel-half) links the two halves of each batch.

Layouts: the host pre-converts everything to f16 and splits src into 4
row/col parity planes, so the stride-2 unfold/fold taps become contiguous
plane windows. q-conv runs per plane (conv1x1 is pointwise), the q.k
reduction uses the custom-DVE TENSOR_TENSOR_REDUCE per tap, and the fold
(a per-channel linear combination of shifted vc planes weighted by E_t)
runs on the TensorEngine as diag(E_t) matmuls accumulated in PSUM. The
final (O*r)*src multiply + f16 output conversion runs on GpSimd reading
PSUM directly; output is written as 4 parity planes the host re-interleaves.
"""

import numpy as np

N_CORES = 8
# per-core src slab (padded): 65 rows x 129 cols, split into parity planes
# P_ab[r, c] = slab[2r+a, 2c+b]
PLANES = [(1, 1), (1, 0), (0, 1), (0, 0)]  # load/compute order: oo, oe, eo, ee
PSHAPE = {(0, 0): (33, 65), (0, 1): (33, 64), (1, 0): (32, 65), (1, 1): (32, 64)}
FEAT_R, FEAT_C = 33, 66         # per-core feat slab (padded, even width)
FEATN = FEAT_R * FEAT_C         # 2178
OUTN = 4 * 2048                 # per-core output: 4 parity planes per channel
SCALE = 1.0 / 64.0

# tap t = 3*i + j reads plane (i%2, j%2) at row off i//2, col off j//2.
# class -> (taps, vc views); fold class (a,b) covers out rows 2y+a cols 2x+b
# vc view key: (dr, dc) -> vc[dr:dr+32, dc:dc+64]
CLS_TAPS = {
    (0, 0): [(4, (0, 0))],
    (0, 1): [(3, (0, 1)), (5, (0, 0))],
    (1, 0): [(1, (1, 0)), (7, (0, 0))],
    (1, 1): [(0, (1, 1)), (2, (1, 0)), (6, (0, 1)), (8, (0, 0))],
}
# fold-src view per class: class (a,b) multiplies src plane (1-a, 1-b)
# at row off a, col off b
TAP_PLANE = {t: ((t // 3) % 2, (t % 3) % 2) for t in range(9)}
TAP_OFF = {t: ((t // 3) // 2, (t % 3) // 2) for t in range(9)}

_prog_cache = {}
TRACE = False
TRACE_KW = {}
LAST_RESULT = [None]


DEBUG = [False]


def _build(add_bv: bool):
    debug = DEBUG[0]
    import concourse.mybir as mybir
    import concourse.tile as tile
    from concourse import bacc
    from concourse.dve_ops import TENSOR_TENSOR_REDUCE

    f32 = mybir.dt.float32
    f16 = mybir.dt.float16
    ADD = mybir.AluOpType.add
    MULT = mybir.AluOpType.mult
    AX = mybir.AxisListType.X
    Exp = mybir.ActivationFunctionType.Exp

    nc = bacc.Bacc("TRN2", target_bir_lowering=False, debug=False,
                   num_devices=N_CORES)

    psz = {ab: PSHAPE[ab][0] * PSHAPE[ab][1] for ab in PLANES}
    src_d = {ab: nc.dram_tensor(f"src_{ab[0]}{ab[1]}", [256, psz[ab]], f16,
                                kind="ExternalInput").ap() for ab in PLANES}
    feat_d = nc.dram_tensor("feat", [256, FEATN], f16, kind="ExternalInput").ap()
    wpack_d = nc.dram_tensor("wpack", [256, 512], f16, kind="ExternalInput").ap()
    sinit_d = nc.dram_tensor("s_init", [256, 9], f32, kind="ExternalInput").ap()
    bv_d = nc.dram_tensor("bv", [256, 1], f32, kind="ExternalInput").ap()
    dones_d = nc.dram_tensor("dones", [128, 128], f16, kind="ExternalInput").ap()
    out_d = nc.dram_tensor("out", [256, OUTN], f16, kind="ExternalOutput").ap()
    if debug:
        dbgS_d = nc.dram_tensor("dbgS", [256, 32], f32, kind="ExternalOutput").ap()
        dbgQ_d = nc.dram_tensor("dbgQ", [256, 9 * 2048], f16,
                                kind="ExternalOutput").ap()
        dbgV_d = nc.dram_tensor("dbgV", [256, FEATN], f16,
                                kind="ExternalOutput").ap()
        dbgO_d = nc.dram_tensor("dbgO", [256, OUTN], f16,
                                kind="ExternalOutput").ap()
        dbgF_d = nc.dram_tensor("dbgF", [256, FEATN], f16,
                                kind="ExternalOutput").ap()
        dbgW_d = nc.dram_tensor("dbgW", [256, 512], f16,
                                kind="ExternalOutput").ap()

    with tile.TileContext(nc) as tc:
        with (
            tc.tile_pool(name="srcp", bufs=2) as srcp,
            tc.tile_pool(name="featp", bufs=2) as featp,
            tc.tile_pool(name="vcp", bufs=2) as vcp,
            tc.tile_pool(name="qcp", bufs=2) as qcp,
            tc.tile_pool(name="constp", bufs=2) as constp,
            tc.tile_pool(name="smp", bufs=1) as smp,
            tc.tile_pool(name="scrp", bufs=1) as scrp,
            tc.tile_pool(name="dgp", bufs=2) as dgp,
            tc.tile_pool(name="outp", bufs=4) as outp,
            tc.tile_pool(name="ps", bufs=2, space="PSUM") as ps,
            tc.tile_pool(name="dramp", bufs=2, space="DRAM") as dramp,
        ):
            # ---- loads: w/dones/smalls, src_oo, feat, then other planes ----
            w_t = []
            for kt in range(2):
                wt = constp.tile([128, 512], f16, tag="w")
                nc.sync.dma_start(wt[:], wpack_d[128 * kt:128 * kt + 128, :])
                w_t.append(wt)
            dones = constp.tile([128, 128], f16, tag="dones")
            nc.sync.dma_start(dones[:], dones_d[:, :])

            # smalls layout (cols):
            # [0:9] S partial  [9:18] Ssum  [18:27] E
            # [27:28] sum  [28:29] r  [30:39] sinit  [40:41] bv
            sm_t = []
            for h in range(2):
                sm = smp.tile([128, 48], f32, tag=f"sm{h}")
                nc.sync.dma_start(sm[:, 30:39], sinit_d[128 * h:128 * h + 128, :])
                if add_bv:
                    nc.sync.dma_start(sm[:, 40:41], bv_d[128 * h:128 * h + 128, :])
                sm_t.append(sm)

            src_t = [{}, {}]    # src_t[h][ab]
            feat_t = []

            def load_plane(ab):
                for h in range(2):
                    st = srcp.tile([128, psz[ab]], f16, tag=f"src{ab}")
                    half = (psz[ab] // 2) & ~63
                    nc.sync.dma_start(st[:, 0:half],
                                      src_d[ab][128 * h:128 * h + 128, 0:half])
                    nc.sync.dma_start(st[:, half:],
                                      src_d[ab][128 * h:128 * h + 128, half:])
                    src_t[h][ab] = st

            for h in range(2):
                ft = featp.tile([128, FEATN], f16, tag="feat")
                nc.sync.dma_start(ft[:, 0:1089], feat_d[128 * h:128 * h + 128, 0:1089])
                nc.sync.dma_start(ft[:, 1089:], feat_d[128 * h:128 * h + 128, 1089:])
                feat_t.append(ft)
            load_plane((1, 1))
            load_plane((1, 0))
            load_plane((0, 1))
            load_plane((0, 0))

            if debug:
                for h in range(2):
                    nc.sync.dma_start(dbgF_d[128 * h:128 * h + 128, :],
                                      feat_t[h][:])
                for kt in range(2):
                    nc.sync.dma_start(dbgW_d[128 * kt:128 * kt + 128, :],
                                      w_t[kt][:])

            # ---- q-conv per parity plane + v-conv + TTR + collectives ----
            S_b, S_r = [], []
            for h in range(2):
                S_b.append(dramp.tile([128, 9], f32, tag=f"sb{h}", name=f"sb{h}"))
                S_r.append(dramp.tile([128, 9], f32, tag=f"sr{h}", name=f"sr{h}"))

            scr = scrp.tile([128, 2048], f16, tag="ttr_scr")
            scr3 = scr.rearrange("p (r q) -> p r q", q=64)
            q_t = []            # q_t[h][ab]
            vc_t = []

            def qconv_plane(h, ab, qtl):
                # Q planes stored at 65-wide row stride so TTR tap views
                # never collapse to 2D (in0/in1 must both stay 3D).
                rows, wid = PSHAPE[ab]
                qt = qcp.tile([128, rows * 65], f16, tag=f"q{ab}")
                qt3 = qt.rearrange("p (r q) -> p r q", q=65)
                for c0 in range(0, psz[ab], 2048):
                    csz = min(2048, psz[ab] - c0)
                    pt = ps.tile([128, 2048], f32, tag="mm")
                    for s0 in range(0, csz, 512):
                        ssz = min(512, csz - s0)
                        for kt in range(2):
                            nc.tensor.matmul(
                                pt[:, s0:s0 + ssz],
                                lhsT=w_t[kt][:, 128 * h:128 * h + 128],
                                rhs=src_t[kt][ab][:, c0 + s0:c0 + s0 + ssz],
                                start=(kt == 0), stop=(kt == 1))
                    if wid == 65:
                        nc.scalar.copy(qt[:, c0:c0 + csz], pt[:, 0:csz])
                    else:
                        nr = csz // 64
                        r0 = c0 // 64
                        nc.scalar.copy(
                            qt3[:, r0:r0 + nr, 0:64],
                            pt.rearrange("p (r q) -> p r q", q=64)[:, 0:nr, :])
                qtl[ab] = qt3

            def vconv(h, evict_eng):
                # evict_eng: 'v' = Vector tensor_scalar, 's' = Scalar copy
                vt = vcp.tile([128, FEATN], f16, tag="vc")
                for c0 in range(0, FEATN, 2048):
                    csz = min(2048, FEATN - c0)
                    pt = ps.tile([128, 2048], f32, tag="mm")
                    for s0 in range(0, csz, 512):
                        ssz = min(512, csz - s0)
                        for kt in range(2):
                            nc.tensor.matmul(
                                pt[:, s0:s0 + ssz],
                                lhsT=w_t[kt][:, 256 + 128 * h:256 + 128 * h + 128],
                                rhs=feat_t[kt][:, c0 + s0:c0 + s0 + ssz],
                                start=(kt == 0), stop=(kt == 1))
                    if evict_eng == 'v':
                        nc.vector.tensor_scalar(
                            out=vt[:, c0:c0 + csz], in0=pt[:, 0:csz],
                            scalar1=sm_t[h][:, 40:41] if add_bv else 1.0,
                            scalar2=None,
                            op0=ADD if add_bv else MULT)
                    elif add_bv:
                        nc.vector.tensor_scalar(
                            out=vt[:, c0:c0 + csz], in0=pt[:, 0:csz],
                            scalar1=sm_t[h][:, 40:41], scalar2=None, op0=ADD)
                    else:
                        nc.scalar.copy(vt[:, c0:c0 + csz], pt[:, 0:csz])
                if add_bv:
                    v3 = vt.rearrange("p (r q) -> p r q", q=FEAT_C)
                    nc.gpsimd.memset(v3[:, :, 64:66], 0.0)
                vc_t.append(vt)

            for h in range(2):
                qtl = {}
                if h == 0:
                    vconv(0, 'v')   # PE warm-up; evict on Vector (idle pre-TTR)
                qconv_plane(h, (1, 1), qtl)
                qconv_plane(h, (1, 0), qtl)
                qconv_plane(h, (0, 1), qtl)
                qconv_plane(h, (0, 0), qtl)
                if h == 0:
                    vconv(1, 's')   # evict woven into Scalar between halves
                q_t.append(qtl)

                # TTR taps in plane-arrival order: oo, oe, eo, ee
                k3 = feat_t[h].rearrange("p (r q) -> p r q", q=FEAT_C)
                sm = sm_t[h]
                for t in [4, 3, 5, 1, 7, 0, 2, 6, 8]:
                    ab = TAP_PLANE[t]
                    r0, c0 = TAP_OFF[t]
                    q3 = qtl[ab]
                    nc.vector._custom_dve(
                        TENSOR_TENSOR_REDUCE,
                        out=scr3[:],
                        in0=q3[:, r0:r0 + 32, c0:c0 + 64],
                        in1=k3[:, 0:32, 0:64],
                        s0=sm[:, 30 + t:31 + t],
                        s1=SCALE,
                        accum_out=sm[:, t:t + 1])
                if debug:
                    for t in range(9):
                        ab = TAP_PLANE[t]
                        r0, c0 = TAP_OFF[t]
                        nc.sync.dma_start(
                            dbgQ_d[128 * h:128 * h + 128,
                                   2048 * t:2048 * t + 2048],
                            qtl[ab][:, r0:r0 + 32, c0:c0 + 64])
                    nc.sync.dma_start(dbgS_d[128 * h:128 * h + 128, 0:9],
                                      sm[:, 0:9])
                nc.sync.dma_start(S_b[h][:], sm[:, 0:9])
                nc.gpsimd.collective_compute(
                    "AllReduce", ADD,
                    replica_groups=[[0, 1], [2, 3], [4, 5], [6, 7]],
                    ins=[S_b[h].opt()], outs=[S_r[h].opt()])

            # ---- E = exp(S), r = 1/sum; fold on PE; (O*r)*src on V/G ----
            for h in range(2):
                sm = sm_t[h]
                nc.sync.dma_start(sm[:, 9:18], S_r[h][:])
                Ev = sm[:, 18:27]
                nc.scalar.activation(Ev, sm[:, 9:18], Exp)
                nc.vector.tensor_reduce(sm[:, 27:28], Ev, axis=AX, op=ADD)
                nc.vector.reciprocal(sm[:, 28:29], sm[:, 27:28])
                if debug:
                    nc.sync.dma_start(dbgS_d[128 * h:128 * h + 128, 9:18],
                                      sm[:, 9:18])
                    nc.sync.dma_start(dbgS_d[128 * h:128 * h + 128, 18:27],
                                      sm[:, 18:27])
                    nc.sync.dma_start(dbgS_d[128 * h:128 * h + 128, 27:29],
                                      sm[:, 27:29])
                    nc.sync.dma_start(dbgV_d[128 * h:128 * h + 128, :],
                                      vc_t[h][:])

                # diag(E_t) tiles for the PE fold. h0's diag-gen + final
                # multiplies avoid Vector (busy with h1's TTRs); h1's run on
                # Vector (idle in the tail, and faster there).
                dg = dgp.tile([128, 9 * 128], f16, tag="diag")
                for t in range(9):
                    if h == 0:
                        nc.scalar.mul(dg[:, 128 * t:128 * t + 128], dones[:],
                                      sm[:, 18 + t:19 + t])
                    else:
                        nc.vector.tensor_scalar(
                            out=dg[:, 128 * t:128 * t + 128], in0=dones[:],
                            scalar1=sm[:, 18 + t:19 + t], scalar2=None, op0=MULT)

                vc3 = vc_t[h].rearrange("p (r q) -> p r q", q=FEAT_C)
                for ci, (a, b) in enumerate(PLANES):  # oo, oe, eo, ee
                    taps = CLS_TAPS[(a, b)]
                    on_v = h == 1 and ci != 1
                    pt = ps.tile([128, 2048], f32, tag="mm")
                    pt3 = pt.rearrange("p (r q) -> p r q", q=64)
                    for ti, (t, (dr, dc)) in enumerate(taps):
                        for r0 in range(0, 32, 8):
                            nc.tensor.matmul(
                                pt3[:, r0:r0 + 8, :],
                                lhsT=dg[:, 128 * t:128 * t + 128],
                                rhs=vc3[:, dr + r0:dr + r0 + 8, dc:dc + 64],
                                start=(ti == 0), stop=(ti == len(taps) - 1))
                    # evict O*r to SBUF f16 (Act, scale=r), then U*src TT.
                    # Vector classes use 65-stride tiles so all TT operands
                    # are matched 3D views (keeps DVE in fast mode).
                    sab = (1 - a, 1 - b)
                    s3 = src_t[h][sab].rearrange("p (r q) -> p r q",
                                                 q=PSHAPE[sab][1])
                    cls = 2 * a + b
                    if on_v:
                        ut = outp.tile([128, 32 * 65], f16, tag="Uv")
                        ut3 = ut.rearrange("p (r q) -> p r q", q=65)
                        nc.scalar.mul(ut3[:, 0:32, 0:64], pt3[:], sm[:, 28:29])
                        ot = outp.tile([128, 32 * 65], f16, tag="Ov")
                        ot3 = ot.rearrange("p (r q) -> p r q", q=65)
                        nc.vector.tensor_tensor(
                            out=ot3[:, 0:32, 0:64], in0=ut3[:, 0:32, 0:64],
                            in1=s3[:, a:a + 32, b:b + 64], op=MULT)
                        nc.sync.dma_start(
                            out_d[128 * h:128 * h + 128,
                                  2048 * cls:2048 * cls + 2048],
                            ot3[:, 0:32, 0:64])
                    else:
                        ut = outp.tile([128, 2048], f16, tag="U")
                        nc.scalar.mul(ut[:], pt[:, 0:2048], sm[:, 28:29])
                        ot = outp.tile([128, 2048], f16, tag="O")
                        nc.gpsimd.tensor_tensor(
                            out=ot.rearrange("p (r q) -> p r q", q=64),
                            in0=ut.rearrange("p (r q) -> p r q", q=64),
                            in1=s3[:, a:a + 32, b:b + 64], op=MULT)
                        nc.sync.dma_start(
                            out_d[128 * h:128 * h + 128,
                                  2048 * cls:2048 * cls + 2048], ot[:])
                    if debug:
                        uview = ut3[:, 0:32, 0:64] if on_v else ut[:]
                        nc.sync.dma_start(
                            dbgO_d[128 * h:128 * h + 128,
                                   2048 * cls:2048 * cls + 2048], uview)

    nc.compile()
    return nc


def _get_program(add_bv: bool):
    if add_bv not in _prog_cache:
        _prog_cache[add_bv] = _build(add_bv)
    return _prog_cache[add_bv]


def kernel(feat, src, Wq, bq, Wv, bv):
    from concourse.bass_utils import run_bass_kernel_spmd

    feat = np.ascontiguousarray(np.asarray(feat, dtype=np.float32))
    src = np.ascontiguousarray(np.asarray(src, dtype=np.float32))
    Wq = np.asarray(Wq, dtype=np.float32)
    bq = np.asarray(bq, dtype=np.float32)
    Wv = np.asarray(Wv, dtype=np.float32)
    bv = np.asarray(bv, dtype=np.float32)
    B, C, H, W = src.shape

    src_pad = np.pad(src, ((0, 0), (0, 0), (1, 1), (1, 1))).astype(np.float16)
    feat_pad = np.pad(feat, ((0, 0), (0, 0), (0, 1), (0, 2))).astype(np.float16)
    wpack = np.ascontiguousarray(
        np.concatenate([Wq.T, Wv.T], axis=1)).astype(np.float16)
    dones = np.eye(128, dtype=np.float16)

    add_bv = bool(np.any(bv))
    nc = _get_program(add_bv)

    # bq correction seeds: S += bq * sum(valid k) * scale
    sinits = {}
    if np.any(bq):
        for b in range(B):
            for s in range(2):
                k = feat[b, :, 32 * s:32 * s + 32, :]
                corr = np.zeros((C, 9), np.float32)
                for i in range(3):
                    for j in range(3):
                        valid = np.ones((32, 64), bool)
                        if i == 0 and s == 0:
                            valid[0, :] = False
                        if j == 0:
                            valid[:, 0] = False
                        corr[:, 3 * i + j] = bq * (k * valid).sum((1, 2)) * SCALE
                sinits[(b, s)] = corr
    zero_sinit = np.zeros((C, 9), np.float32)

    in_maps = []
    for core in range(N_CORES):
        b, s = core // 2, core % 2
        slab = src_pad[b, :, 64 * s:64 * s + 65, :129]
        im = {
            "feat": np.ascontiguousarray(
                feat_pad[b, :, 32 * s:32 * s + FEAT_R, :FEAT_C]
            ).reshape(C, FEATN),
            "wpack": wpack,
            "s_init": sinits.get((b, s), zero_sinit),
            "bv": bv.reshape(C, 1).astype(np.float32),
            "dones": dones,
        }
        for (a, bb) in PLANES:
            pr, pc = PSHAPE[(a, bb)]
            im[f"src_{a}{bb}"] = np.ascontiguousarray(
                slab[:, a::2, bb::2]).reshape(C, pr * pc)
        in_maps.append(im)

    res = run_bass_kernel_spmd(nc, in_maps, list(range(N_CORES)),
                               trace=TRACE, **TRACE_KW)
    LAST_RESULT[0] = res

    out = np.empty((B, C, H, W), np.float32)
    for core in range(N_CORES):
        b, s = core // 2, core % 2
        planes = res.results[core]["out"].astype(np.float32).reshape(C, 4, 32, 64)
        for cls, (a, bb) in enumerate([(0, 0), (0, 1), (1, 0), (1, 1)]):
            out[b, :, 64 * s + a:64 * s + 64:2, bb::2] = planes[:, cls]
    return out
